# revision 27
# baseline (speedup 1.0000x reference)
"""Trainium2 Bass kernel for nn_Attention_90744069030375.

Reference computation (per batch b, S=2048, D=1024):
    scores = (q @ k^T) * scale                      [S, S]
    attn_mask = max(pad_i, pad_j, causal_triu)      (pad = ~mask)
    scores -= 1e9 * attn_mask
    attn   = softmax(scores, -1)
    out    = attn @ k        (v = k)

Two numerical subtleties drive the design:

1. For a padded query row (mask[i]=False) every logit gets -1e9, which
   *mathematically* cancels in softmax -- but in fp32 ulp(1e9) = 64, so
   `scores - 1e9` collapses the row onto a 64-wide grid and softmax becomes
   uniform over the top bucket.  The grading reference runs in fp32 and has
   exactly this behavior, so we reproduce it: the additive bias is shipped
   as an exact fp32 {0, -1e9} tensor and applied to fp32 scores.

2. Bucket membership flips if our scores differ from the reference's by
   more than ~ulp-boundary distances, so QK^T must be near-fp32-accurate.
   A single fp16 pass (logit err ~1.6e-2) fails; a bf16 hi/lo 3-pass
   (q ~ qh+ql, k ~ kh+kl, scores = qh.kh + qh.kl + ql.kh, fp32 PSUM
   accumulation, logit err ~1e-4) gives aggregate rel err ~2e-4.

Sharding: data-parallel over batch -- 8 batches -> 8 NeuronCores, one
batch each, no collectives.  Host pre-marshals per core: transposed bf16
hi/lo q and k ([D,S], lhsT/rhs for QK^T), fp16 k ([S,D], rhs for attn@K),
and the fp32 mask bias ([S,S], streamed per row-tile).  Softmax runs on
ACT (exp with fused row-sum via accum_out) + DVE (max/reciprocal/bias
add).  attn is transposed for the PV matmul with the DMA xbar transpose
(SBUF->SBUF fp16); PV accumulates over 16 key blocks into fp32 PSUM and
rows are scaled by 1/rowsum on the way out.
"""

import numpy as np
import ml_dtypes

import concourse.bass as bass
import concourse.bacc as bacc
import concourse.mybir as mybir
from concourse.bass_utils import run_bass_kernel_spmd
from concourse.tile import TileContext

B, S, D = 8, 2048, 1024
P = 128                 # partitions / M-tile rows
NQ = S // P             # 16 query row-tiles
ND = D // P             # 8 contraction tiles for QK^T
NJ = S // 512           # 4 key column tiles of 512
BF16 = mybir.dt.bfloat16
F16 = mybir.dt.float16
F32 = mybir.dt.float32


def build_bass(reps=1, qk_order="d", sc_bufs=6, pv_bufs=2):
    nc = bacc.Bacc()
    qTh = nc.dram_tensor("qTh", [D, S], BF16, kind="ExternalInput")
    qTl = nc.dram_tensor("qTl", [D, S], BF16, kind="ExternalInput")
    kTh = nc.dram_tensor("kTh", [D, S], BF16, kind="ExternalInput")
    kTl = nc.dram_tensor("kTl", [D, S], BF16, kind="ExternalInput")
    kpv = nc.dram_tensor("kpv", [S, D], F16, kind="ExternalInput")
    masku8 = nc.dram_tensor("masku8", [S, S], mybir.dt.uint8,
                            kind="ExternalInput")
    out = nc.dram_tensor("out", [S, D], F32, kind="ExternalOutput")

    with TileContext(nc) as tc:
        with (
            tc.tile_pool(name="weights", bufs=1) as wpool,
            tc.tile_pool(name="work", bufs=2) as work,
            tc.tile_pool(name="stats", bufs=3) as stats,
            tc.tile_pool(name="scores", bufs=sc_bufs, space="PSUM") as scores_pool,
            tc.tile_pool(name="pv", bufs=pv_bufs, space="PSUM") as pv_pool,
        ):
            # ---- persistent operands (merged tiles: one slot per group) --
            # [:, d*S:(d+1)*S] of qTh_all is the [128, S] d-th contraction
            # tile of q-hi, etc.  Loads are issued on the SP HWDGE queues in
            # the order the matmul loop consumes them (k column-chunks in n
            # order first) so the first banks can start after ~4MB instead
            # of ~20MB.  The xbar attn transposes live on the ACT HWDGE
            # queues instead (xbar-mode transitions serialize per queue).
            qTh_all = wpool.tile([P, ND * S], BF16, tag="qTh")
            qTl_all = wpool.tile([P, ND * S], BF16, tag="qTl")
            kTh_all = wpool.tile([P, ND * S], BF16, tag="kTh")
            kTl_all = wpool.tile([P, ND * S], BF16, tag="kTl")
            kpv_all = wpool.tile([P, NQ * D], F16, tag="kpv")
            for n in range(NJ):
                nsl = slice(n * 512, (n + 1) * 512)
                for d in range(ND):
                    sl = slice(d * P, (d + 1) * P)
                    nc.sync.dma_start(
                        out=kTh_all[:, d * S:(d + 1) * S][:, nsl],
                        in_=kTh[sl, nsl])
                    nc.sync.dma_start(
                        out=kTl_all[:, d * S:(d + 1) * S][:, nsl],
                        in_=kTl[sl, nsl])
                    if n == 0:
                        # q slices for the first few row-tiles, interleaved
                        # with kT n=0 so the d-th matmul of the first bank
                        # can start as soon as its own d-chunks land
                        nc.sync.dma_start(
                            out=qTh_all[:, d * S:(d + 1) * S][:, 0:512],
                            in_=qTh[sl, 0:512])
                        nc.sync.dma_start(
                            out=qTl_all[:, d * S:(d + 1) * S][:, 0:512],
                            in_=qTl[sl, 0:512])
            for j in range(NQ):
                nc.sync.dma_start(out=kpv_all[:, j * D:(j + 1) * D],
                                  in_=kpv[j * P:(j + 1) * P, :])
            for d in range(ND):
                sl = slice(d * P, (d + 1) * P)
                nc.sync.dma_start(
                    out=qTh_all[:, d * S:(d + 1) * S][:, 512:S],
                    in_=qTh[sl, 512:S])
                nc.sync.dma_start(
                    out=qTl_all[:, d * S:(d + 1) * S][:, 512:S],
                    in_=qTl[sl, 512:S])

            # ---- main loop over query row-tiles --------------------------
            # reps>1 repeats the whole computation back-to-back in one NEFF
            # (benchmarking only: marginal time per rep = steady-state time)
            for m_rep in range(reps * NQ):
                m = m_rep % NQ
                # mask rows stream as u8; expand to the exact fp32 {0,-1e9}
                # additive bias on DVE (16MB -> 2MB of DMA)
                bias_u8 = work.tile([P, S], mybir.dt.uint8, tag="bias_u8")
                nc.gpsimd.dma_start(
                    out=bias_u8, in_=masku8[m * P:(m + 1) * P, :])
                bias = work.tile([P, S], F32, tag="bias")
                nc.vector.tensor_scalar_mul(bias, bias_u8, float(-1e9))

                # QK^T: 3 bf16 passes accumulate in fp32 PSUM; n-outer so
                # each bank finishes early and softmax pipelines behind the
                # next bank's matmuls.
                msl = slice(m * P, (m + 1) * P)
                nsls = [slice(n * 512, (n + 1) * 512) for n in range(NJ)]
                sc = [scores_pool.tile([P, 512], F32, name=f"sc{n}", tag="sc")
                      for n in range(NJ)]
                pmax = stats.tile([P, NJ], F32, tag="pmax")
                # qk_order: how many PSUM banks share one LDWEIGHTS of the
                # q slice -- "n": 1 bank (64 LDW/m, per-bank completion),
                # "pair": 2 banks (32 LDW/m), "d": all 4 banks (16 LDW/m,
                # banks complete together).
                group = {"n": 1, "pair": 2, "d": NJ}[qk_order]
                for g0 in range(0, NJ, group):
                    ns = range(g0, min(g0 + group, NJ))
                    for d in range(ND):
                        qh_d = qTh_all[:, d * S:(d + 1) * S][:, msl]
                        for n in ns:
                            nc.tensor.matmul(t := sc[n], qh_d,
                                             kTh_all[:, d * S:(d + 1) * S][:, nsls[n]],
                                             start=(d == 0), stop=False)
                            nc.tensor.matmul(t, qh_d,
                                             kTl_all[:, d * S:(d + 1) * S][:, nsls[n]],
                                             start=False, stop=False)
                    for d in range(ND):
                        ql_d = qTl_all[:, d * S:(d + 1) * S][:, msl]
                        for n in ns:
                            nc.tensor.matmul(sc[n], ql_d,
                                             kTh_all[:, d * S:(d + 1) * S][:, nsls[n]],
                                             start=False, stop=(d == ND - 1))
                    for n in ns:
                        # exact fp32 reference bias (in-place on PSUM, DVE)
                        nc.vector.tensor_add(sc[n], sc[n], bias[:, nsls[n]])
                        nc.vector.reduce_max(
                            pmax[:, n:n + 1], sc[n], axis=mybir.AxisListType.X)

                negmax = stats.tile([P, 1], F32, tag="negmax")
                nc.vector.reduce_max(
                    negmax, pmax, axis=mybir.AxisListType.X, negate=True)

                # exp(x - rowmax) on ACT, row-sums fused via accum_out
                attn = work.tile([P, S], F16, tag="attn")
                psums = stats.tile([P, NJ], F32, tag="psums")
                for n in range(NJ):
                    nc.scalar.activation(
                        out=attn[:, n * 512:(n + 1) * 512],
                        in_=sc[n],
                        func=mybir.ActivationFunctionType.Exp,
                        bias=negmax,
                        scale=1.0,
                        accum_out=psums[:, n:n + 1],
                    )
                recip = stats.tile([P, 1], F32, tag="recip")
                nc.vector.reduce_sum(recip, psums, axis=mybir.AxisListType.X)
                nc.vector.reciprocal(recip, recip)

                # transpose attn for PV (DMA xbar): attnT[:, jb, :] is the
                # [j=128, i=128] lhsT block for key block jb
                attnT = work.tile([P, NQ, P], F16, tag="attnT")
                for n in range(NJ):
                    nc.scalar.dma_start(
                        out=attnT[:, 4 * n:4 * (n + 1), :],
                        in_=attn[:, n * 512:(n + 1) * 512],
                        transpose=True,
                    )

                # PV: out[i, d] += attnT[:, jb]^T @ kpv[jb]
                pv = [pv_pool.tile([P, 512], F32, name=f"pv{nn}", tag="pv")
                      for nn in range(2)]
                for jb in range(NQ):
                    lhsT = attnT[:, jb, :]
                    for nn in range(2):
                        nc.tensor.matmul(
                            pv[nn],
                            lhsT,
                            kpv_all[:, jb * D:(jb + 1) * D][
                                :, nn * 512:(nn + 1) * 512],
                            start=(jb == 0),
                            stop=(jb == NQ - 1),
                        )

                # normalize rows and store
                osb = work.tile([P, D], F32, tag="osb")
                for nn in range(2):
                    nc.vector.tensor_scalar_mul(
                        osb[:, nn * 512:(nn + 1) * 512], pv[nn], recip)
                nc.sync.dma_start(
                    out=out[m * P:(m + 1) * P, :], in_=osb)

    return nc


_NC_CACHE = None


def _get_nc():
    global _NC_CACHE
    if _NC_CACHE is None:
        _NC_CACHE = build_bass()
        if not _NC_CACHE.is_finalized():
            _NC_CACHE.finalize()
    return _NC_CACHE


def make_in_maps(q, k, mask, scale):
    bf = ml_dtypes.bfloat16
    triu = np.triu(np.ones((S, S), np.float32), k=1)
    in_maps = []
    s = float(np.asarray(scale))
    for b in range(B):
        qs = (q[b] * s).astype(np.float32)
        qh = qs.astype(bf)
        ql = (qs - qh.astype(np.float32)).astype(bf)
        kh = k[b].astype(bf)
        kl = (k[b] - kh.astype(np.float32)).astype(bf)
        pad = (~mask[b]).astype(np.float32)
        am = np.maximum(np.maximum(pad[:, None], pad[None, :]), triu)
        in_maps.append({
            "qTh": np.ascontiguousarray(qh.T),
            "qTl": np.ascontiguousarray(ql.T),
            "kTh": np.ascontiguousarray(kh.T),
            "kTl": np.ascontiguousarray(kl.T),
            "kpv": np.ascontiguousarray(k[b].astype(np.float16)),
            "masku8": am.astype(np.uint8),
        })
    return in_maps


def kernel(q, k, mask, scale, _want_trace=False, **trace_kwargs):
    nc = _get_nc()
    in_maps = make_in_maps(
        np.asarray(q), np.asarray(k), np.asarray(mask), np.asarray(scale))
    res = run_bass_kernel_spmd(
        nc, in_maps, list(range(B)), trace=_want_trace, **trace_kwargs)
    outs = np.stack([res.results[i]["out"] for i in range(B)], axis=0)
    outs = outs.astype(np.float32)
    if _want_trace:
        return outs, res
    return outs


# revision 31
# speedup vs baseline: 1.0001x; 1.0001x over previous
"""Trainium2 Bass kernel for nn_Attention_90744069030375.

Reference computation (per batch b, S=2048, D=1024):
    scores = (q @ k^T) * scale                      [S, S]
    attn_mask = max(pad_i, pad_j, causal_triu)      (pad = ~mask)
    scores -= 1e9 * attn_mask
    attn   = softmax(scores, -1)
    out    = attn @ k        (v = k)

Two numerical subtleties drive the design:

1. For a padded query row (mask[i]=False) every logit gets -1e9, which
   *mathematically* cancels in softmax -- but in fp32 ulp(1e9) = 64, so
   `scores - 1e9` collapses the row onto a 64-wide grid and softmax becomes
   uniform over the top bucket.  The grading reference runs in fp32 and has
   exactly this behavior, so we reproduce it: the additive bias is shipped
   as an exact fp32 {0, -1e9} tensor and applied to fp32 scores.

2. Bucket membership flips if our scores differ from the reference's by
   more than ~ulp-boundary distances, so QK^T must be near-fp32-accurate.
   A single fp16 pass (logit err ~1.6e-2) fails; a bf16 hi/lo 3-pass
   (q ~ qh+ql, k ~ kh+kl, scores = qh.kh + qh.kl + ql.kh, fp32 PSUM
   accumulation, logit err ~1e-4) gives aggregate rel err ~2e-4.

Sharding: data-parallel over batch -- 8 batches -> 8 NeuronCores, one
batch each, no collectives.  Host pre-marshals per core: transposed bf16
hi/lo q and k ([D,S], lhsT/rhs for QK^T), fp16 k ([S,D], rhs for attn@K),
and the fp32 mask bias ([S,S], streamed per row-tile).  Softmax runs on
ACT (exp with fused row-sum via accum_out) + DVE (max/reciprocal/bias
add).  attn is transposed for the PV matmul with the DMA xbar transpose
(SBUF->SBUF fp16); PV accumulates over 16 key blocks into fp32 PSUM and
rows are scaled by 1/rowsum on the way out.
"""

import numpy as np
import ml_dtypes

import concourse.bass as bass
import concourse.bacc as bacc
import concourse.mybir as mybir
from concourse.bass_utils import run_bass_kernel_spmd
from concourse.tile import TileContext

B, S, D = 8, 2048, 1024
P = 128                 # partitions / M-tile rows
NQ = S // P             # 16 query row-tiles
ND = D // P             # 8 contraction tiles for QK^T
NJ = S // 512           # 4 key column tiles of 512
BF16 = mybir.dt.bfloat16
F16 = mybir.dt.float16
F32 = mybir.dt.float32


def build_bass(reps=1, qk_order="d", sc_bufs=6, pv_bufs=2):
    nc = bacc.Bacc()
    qTh = nc.dram_tensor("qTh", [D, S], BF16, kind="ExternalInput")
    qTl = nc.dram_tensor("qTl", [D, S], BF16, kind="ExternalInput")
    kTh = nc.dram_tensor("kTh", [D, S], BF16, kind="ExternalInput")
    kTl = nc.dram_tensor("kTl", [D, S], BF16, kind="ExternalInput")
    kpv = nc.dram_tensor("kpv", [S, D], F16, kind="ExternalInput")
    masku8 = nc.dram_tensor("masku8", [S, S], mybir.dt.uint8,
                            kind="ExternalInput")
    out = nc.dram_tensor("out", [S, D], F32, kind="ExternalOutput")

    with TileContext(nc) as tc:
        with (
            tc.tile_pool(name="weights", bufs=1) as wpool,
            tc.tile_pool(name="work", bufs=2) as work,
            tc.tile_pool(name="stats", bufs=3) as stats,
            tc.tile_pool(name="scores", bufs=sc_bufs, space="PSUM") as scores_pool,
            tc.tile_pool(name="pv", bufs=pv_bufs, space="PSUM") as pv_pool,
        ):
            # ---- persistent operands (merged tiles: one slot per group) --
            # [:, d*S:(d+1)*S] of qTh_all is the [128, S] d-th contraction
            # tile of q-hi, etc.  Loads are issued on the SP HWDGE queues in
            # the order the matmul loop consumes them (k column-chunks in n
            # order first) so the first banks can start after ~4MB instead
            # of ~20MB.  The xbar attn transposes live on the ACT HWDGE
            # queues instead (xbar-mode transitions serialize per queue).
            qTh_all = wpool.tile([P, ND * S], BF16, tag="qTh")
            qTl_all = wpool.tile([P, ND * S], BF16, tag="qTl")
            kTh_all = wpool.tile([P, ND * S], BF16, tag="kTh")
            kTl_all = wpool.tile([P, ND * S], BF16, tag="kTl")
            kpv_all = wpool.tile([P, NQ * D], F16, tag="kpv")
            for n in range(NJ):
                nsl = slice(n * 512, (n + 1) * 512)
                for d in range(ND):
                    sl = slice(d * P, (d + 1) * P)
                    nc.sync.dma_start(
                        out=kTh_all[:, d * S:(d + 1) * S][:, nsl],
                        in_=kTh[sl, nsl])
                    nc.sync.dma_start(
                        out=kTl_all[:, d * S:(d + 1) * S][:, nsl],
                        in_=kTl[sl, nsl])
                    if n == 0:
                        # q slices for the first few row-tiles, interleaved
                        # with kT n=0 so the d-th matmul of the first bank
                        # can start as soon as its own d-chunks land
                        nc.sync.dma_start(
                            out=qTh_all[:, d * S:(d + 1) * S][:, 0:512],
                            in_=qTh[sl, 0:512])
                        nc.sync.dma_start(
                            out=qTl_all[:, d * S:(d + 1) * S][:, 0:512],
                            in_=qTl[sl, 0:512])
            for j in range(NQ):
                nc.sync.dma_start(out=kpv_all[:, j * D:(j + 1) * D],
                                  in_=kpv[j * P:(j + 1) * P, :])
            for d in range(ND):
                sl = slice(d * P, (d + 1) * P)
                nc.sync.dma_start(
                    out=qTh_all[:, d * S:(d + 1) * S][:, 512:S],
                    in_=qTh[sl, 512:S])
                nc.sync.dma_start(
                    out=qTl_all[:, d * S:(d + 1) * S][:, 512:S],
                    in_=qTl[sl, 512:S])

            # ---- main loop over query row-tiles --------------------------
            pending_pv = []
            # reps>1 repeats the whole computation back-to-back in one NEFF
            # (benchmarking only: marginal time per rep = steady-state time)
            for m_rep in range(reps * NQ):
                m = m_rep % NQ
                # mask rows stream as u8; expand to the exact fp32 {0,-1e9}
                # additive bias on DVE (16MB -> 2MB of DMA)
                bias_u8 = work.tile([P, S], mybir.dt.uint8, tag="bias_u8")
                nc.gpsimd.dma_start(
                    out=bias_u8, in_=masku8[m * P:(m + 1) * P, :])
                bias = work.tile([P, S], F32, tag="bias")
                nc.vector.tensor_scalar_mul(bias, bias_u8, float(-1e9))

                # QK^T: 3 bf16 passes accumulate in fp32 PSUM; n-outer so
                # each bank finishes early and softmax pipelines behind the
                # next bank's matmuls.
                msl = slice(m * P, (m + 1) * P)
                nsls = [slice(n * 512, (n + 1) * 512) for n in range(NJ)]
                sc = [scores_pool.tile([P, 512], F32, name=f"sc{n}", tag="sc")
                      for n in range(NJ)]
                pmax = stats.tile([P, NJ], F32, tag="pmax")
                # qk_order: how many PSUM banks share one LDWEIGHTS of the
                # q slice -- "n": 1 bank (64 LDW/m, per-bank completion),
                # "pair": 2 banks (32 LDW/m), "d": all 4 banks (16 LDW/m,
                # banks complete together).
                group = {"n": 1, "pair": 2, "d": NJ}[qk_order]
                for g0 in range(0, NJ, group):
                    ns = range(g0, min(g0 + group, NJ))
                    for d in range(ND):
                        qh_d = qTh_all[:, d * S:(d + 1) * S][:, msl]
                        for n in ns:
                            nc.tensor.matmul(t := sc[n], qh_d,
                                             kTh_all[:, d * S:(d + 1) * S][:, nsls[n]],
                                             start=(d == 0), stop=False)
                            nc.tensor.matmul(t, qh_d,
                                             kTl_all[:, d * S:(d + 1) * S][:, nsls[n]],
                                             start=False, stop=False)
                    for d in range(ND):
                        ql_d = qTl_all[:, d * S:(d + 1) * S][:, msl]
                        for n in ns:
                            nc.tensor.matmul(sc[n], ql_d,
                                             kTh_all[:, d * S:(d + 1) * S][:, nsls[n]],
                                             start=False, stop=(d == ND - 1))
                    for n in ns:
                        # exact fp32 reference bias (in-place on PSUM, DVE)
                        nc.vector.tensor_add(sc[n], sc[n], bias[:, nsls[n]])
                        nc.vector.reduce_max(
                            pmax[:, n:n + 1], sc[n], axis=mybir.AxisListType.X)

                negmax = stats.tile([P, 1], F32, tag="negmax")
                nc.vector.reduce_max(
                    negmax, pmax, axis=mybir.AxisListType.X, negate=True)

                # exp(x - rowmax) on ACT, row-sums fused via accum_out
                attn = work.tile([P, S], F16, tag="attn")
                psums = stats.tile([P, NJ], F32, tag="psums")
                for n in range(NJ):
                    nc.scalar.activation(
                        out=attn[:, n * 512:(n + 1) * 512],
                        in_=sc[n],
                        func=mybir.ActivationFunctionType.Exp,
                        bias=negmax,
                        scale=1.0,
                        accum_out=psums[:, n:n + 1],
                    )
                recip = stats.tile([P, 1], F32, tag="recip")
                nc.vector.reduce_sum(recip, psums, axis=mybir.AxisListType.X)
                nc.vector.reciprocal(recip, recip)

                # transpose attn for PV (DMA xbar): attnT[:, jb, :] is the
                # [j=128, i=128] lhsT block for key block jb
                attnT = work.tile([P, NQ, P], F16, tag="attnT", bufs=3)
                for n in range(NJ):
                    nc.scalar.dma_start(
                        out=attnT[:, 4 * n:4 * (n + 1), :],
                        in_=attn[:, n * 512:(n + 1) * 512],
                        transpose=True,
                    )

                # PV is emitted AFTER the next tile's QK^T (deferred
                # closure): both PV(m) and QK(m+1) gate on softmax(m), and
                # with PV(m) at lower scheduler priority it stays available
                # to fill the softmax latency of the FINAL tile, which
                # otherwise leaves the PE idle ~10us at the kernel tail.
                def make_pv(m, attnT, recip):
                    def emit_pv():
                        pv = [pv_pool.tile([P, 512], F32, name=f"pv{nn}",
                                           tag="pv") for nn in range(2)]
                        for jb in range(NQ):
                            lhsT = attnT[:, jb, :]
                            for nn in range(2):
                                nc.tensor.matmul(
                                    pv[nn],
                                    lhsT,
                                    kpv_all[:, jb * D:(jb + 1) * D][
                                        :, nn * 512:(nn + 1) * 512],
                                    start=(jb == 0),
                                    stop=(jb == NQ - 1),
                                )
                        # normalize rows and store
                        osb = work.tile([P, D], F32, name="osb", tag="osb", bufs=1)
                        for nn in range(2):
                            nc.vector.tensor_scalar_mul(
                                osb[:, nn * 512:(nn + 1) * 512], pv[nn],
                                recip)
                        nc.sync.dma_start(
                            out=out[m * P:(m + 1) * P, :], in_=osb)
                    return emit_pv

                if len(pending_pv) == 2:
                    pending_pv.pop(0)()
                pending_pv.append(make_pv(m, attnT, recip))
            for f in pending_pv:
                f()

    return nc


_NC_CACHE = None


def _get_nc():
    global _NC_CACHE
    if _NC_CACHE is None:
        _NC_CACHE = build_bass()
        if not _NC_CACHE.is_finalized():
            _NC_CACHE.finalize()
    return _NC_CACHE


def make_in_maps(q, k, mask, scale):
    bf = ml_dtypes.bfloat16
    triu = np.triu(np.ones((S, S), np.float32), k=1)
    in_maps = []
    s = float(np.asarray(scale))
    for b in range(B):
        qs = (q[b] * s).astype(np.float32)
        qh = qs.astype(bf)
        ql = (qs - qh.astype(np.float32)).astype(bf)
        kh = k[b].astype(bf)
        kl = (k[b] - kh.astype(np.float32)).astype(bf)
        pad = (~mask[b]).astype(np.float32)
        am = np.maximum(np.maximum(pad[:, None], pad[None, :]), triu)
        in_maps.append({
            "qTh": np.ascontiguousarray(qh.T),
            "qTl": np.ascontiguousarray(ql.T),
            "kTh": np.ascontiguousarray(kh.T),
            "kTl": np.ascontiguousarray(kl.T),
            "kpv": np.ascontiguousarray(k[b].astype(np.float16)),
            "masku8": am.astype(np.uint8),
        })
    return in_maps


def kernel(q, k, mask, scale, _want_trace=False, **trace_kwargs):
    nc = _get_nc()
    in_maps = make_in_maps(
        np.asarray(q), np.asarray(k), np.asarray(mask), np.asarray(scale))
    res = run_bass_kernel_spmd(
        nc, in_maps, list(range(B)), trace=_want_trace, **trace_kwargs)
    outs = np.stack([res.results[i]["out"] for i in range(B)], axis=0)
    outs = outs.astype(np.float32)
    if _want_trace:
        return outs, res
    return outs
